# revision 20
# baseline (speedup 1.0000x reference)
"""Stein solver  Lambda - A @ Lambda @ W = C @ Y  on 8 trn2 NeuronCores.

Math: Lambda = sum_k A^k R W^k with R = C@Y; contraction ||A||2*||W||2 ~ 0.32.
Smith doubling truncated at 4 terms, then 2 exact fixed-point iterations:
    S1 = R + (A C) Y W           (2 terms; U0 = (A C) Y avoids gathering R)
    S2 = S1 + A^2 S1 W^2         (4 terms; ~1%-magnitude term, bf16 GEMMs)
    Sp = R + A S W   (x2, fp32)  (polish: each contracts error ~12x -> ~5e-7)

Distribution: row-sharded over 8 cores, core c owns rows [128c, 128c+128).
Stationary operand = transposed own-shard (8 k-tiles of [128,128]); moving
operand = the full matrix, streamed from DRAM -- except W, which stays
resident in SBUF (64KB/partition) and feeds three cgemms for free.
AllGathers carry only what later phases read as full matrices: S1 (bf16,
hidden behind the A^2/W^2 cgemms), W^2 (bf16), S2 (fp32), Sp (fp32).

Precision: fp32 GEMMs (4 cyc/row) everywhere on the main chain -- the fp32
phases are DMA-bound anyway, so the extra PE cycles are free and keep the
PE warm; bf16 GEMMs (1 cyc/row, half DMA) for the 1%-scale S2 update, whose
error is wiped by the polish.  Complex GEMM = 4 real GEMMs with the real-part
subtraction folded into PSUM accumulation via pre-negated imag weights.
"""

import numpy as np

P = 128
N = 1024
KT = N // P          # 8 k-tiles
NC = 8               # cores
NCH = 2              # 512-wide n-chunks per 1024-col output row block

_compiled = {}


def _build():
    import concourse.mybir as mybir
    import concourse.tile as tile
    from concourse import bacc
    from concourse.masks import make_identity

    f32 = mybir.dt.float32
    bf16 = mybir.dt.bfloat16

    nc = bacc.Bacc("TRN2", target_bir_lowering=False, debug=False, num_devices=NC)

    # ---- I/O ----  full matrices laid out [partition, plane, ktile, col]:
    # X[kt*128+p, c] at [p, j, kt, c]; shards [partition, (re,im,-im), ktile, m]
    Csp = nc.dram_tensor("Csp", [P, 4, KT, N], bf16, kind="ExternalInput")
    Ysp = nc.dram_tensor("Ysp", [P, 4, KT, N], bf16, kind="ExternalInput")
    Wsp_d = nc.dram_tensor("Wsp_d", [P, 4, KT, N], bf16, kind="ExternalInput")
    Yfull32 = nc.dram_tensor("Yfull32", [P, 2, KT, N], f32, kind="ExternalInput")
    Afull_bf = nc.dram_tensor("Afull_bf", [P, 2, KT, N], bf16, kind="ExternalInput")
    Wfull_bf = nc.dram_tensor("Wfull_bf", [P, 2, KT, N], bf16, kind="ExternalInput")
    ATsp = nc.dram_tensor("ATsp", [P, 6, KT, P], bf16, kind="ExternalInput")
    ATsh_bf = nc.dram_tensor("ATsh_bf", [P, 3, KT, P], bf16, kind="ExternalInput")
    WTsh_bf = nc.dram_tensor("WTsh_bf", [P, 3, KT, P], bf16, kind="ExternalInput")
    CTsh32 = nc.dram_tensor("CTsh32", [P, 3, KT, P], f32, kind="ExternalInput")
    out = nc.dram_tensor("out", [2, P, N], f32, kind="ExternalOutput")

    RG = [list(range(NC))]

    with tile.TileContext(nc) as tc:
        with (
            tc.tile_pool(name="wpin", bufs=1) as wpin,        # pinned
            tc.tile_pool(name="wrot", bufs=2) as wrot,        # rotating weights
            tc.tile_pool(name="rhs", bufs=2) as rpool,        # rhs stream tiles
            tc.tile_pool(name="acc", bufs=2) as apool,        # shard accumulators
            tc.tile_pool(name="psum", bufs=6, space="PSUM") as ppool,
            tc.tile_pool(name="tpsum", bufs=2, space="PSUM") as tppool,
            tc.tile_pool(name="dram", bufs=1, space="DRAM") as dram,
        ):
            ident = wpin.tile([P, P], f32, tag="ident")
            make_identity(nc, ident)
            ident_bf = wpin.tile([P, P], bf16, tag="identbf")
            nc.vector.tensor_copy(ident_bf[:], ident[:])

            # W (split hi/lo bf16) stays SBUF-resident: feeds S1/Sp cgemms.
            Wsp = wpin.tile([P, 4, KT, N], bf16, tag="Wsp")
            nc.sync.dma_start(Wsp[:], Wsp_d.ap())
            ATspw = wpin.tile([P, 6, KT, P], bf16, tag="ATspw")
            nc.sync.dma_start(ATspw[:], ATsp.ap())

            def load_weights(dram_t, tag, dtype, pool=wrot, bufs=None):
                wt = pool.tile([P, 3, KT, P], dtype, tag=tag, name="wt_" + tag,
                               bufs=bufs)
                nc.sync.dma_start(wt[:], dram_t.ap())
                return wt

            def cgemm(XT, rhs_slice, out_cb, dtype, resident=None, halves=None):
                """out(own 128 rows x 1024, complex) = own_rows(X) @ M.

                XT: [P,3,KT,P] weights (re, im, -im).  Moving operand: either
                rhs_slice(j,t) -> DRAM [P,N] AP (streamed via SBUF tiles) or
                resident(j,t) -> SBUF [P,N] AP.  out_cb(j, ci, psum).
                """
                ps = [[ppool.tile([P, 512], f32, tag="ps", name="ps")
                       for _ in range(NCH)] for _ in range(2)]

                def do4(ci, rsl, cs, st, sp):
                    nc.tensor.matmul(ps[0][ci][:], XT[:, 0, t], rsl(0, cs), start=st, stop=False)
                    nc.tensor.matmul(ps[0][ci][:], XT[:, 2, t], rsl(1, cs), start=False, stop=sp)
                    nc.tensor.matmul(ps[1][ci][:], XT[:, 0, t], rsl(1, cs), start=st, stop=False)
                    nc.tensor.matmul(ps[1][ci][:], XT[:, 1, t], rsl(0, cs), start=False, stop=sp)

                if halves is not None:
                    for ci in range(NCH):
                        for t in range(KT):
                            rt = rpool.tile([P, 2, 512], dtype, tag="rhsh", name="rth", bufs=2)
                            nc.sync.dma_start(rt[:], halves[ci](t))
                            do4(ci, (lambda j, cs, rt=rt: rt[:, j, :]), None, t == 0, t == KT - 1)
                else:
                    for t in range(KT):
                        if resident is None:
                            rt = rpool.tile([P, 2, N], dtype, tag="rhs", name="rt")
                            nc.sync.dma_start(rt[:, 0], rhs_slice(0, t))
                            nc.sync.dma_start(rt[:, 1], rhs_slice(1, t))
                            rsl = lambda j, cs: rt[:, j, cs]
                        else:
                            rsl = lambda j, cs, t=t: resident(j, t)[:, cs]
                        st = t == 0
                        sp = t == KT - 1
                        for ci in range(NCH):
                            cs = slice(512 * ci, 512 * ci + 512)
                            do4(ci, rsl, cs, st, sp)
                for j in range(2):
                    for ci in range(NCH):
                        out_cb(j, ci, ps[j][ci])

            def cgemm_sp(XT6, rhs_slice, out_cb, resident=None, halves=None):
                """Split-bf16 complex GEMM: X, M given as hi/lo bf16 pairs.

                XT6: [P,6,KT,P] weights (rh, rl, ih, il, -ih, -il).
                rhs_slice(t) -> DRAM [P,4,N] AP (planes rh, rl, ih, il), or
                resident(y, t) -> SBUF [P,N] AP, or halves[ci](t) -> [P,4,512]
                column-half APs (chunk-outer order, for pipelined AllGathers).
                """
                MMS_R = ((0, 0), (0, 1), (1, 0), (4, 2), (4, 3), (5, 2))
                MMS_I = ((0, 2), (0, 3), (1, 2), (2, 0), (2, 1), (3, 0))
                ps = [[ppool.tile([P, 512], f32, tag="ps", name="ps")
                       for _ in range(NCH)] for _ in range(2)]

                def do_mms(ci, rsl, cs, st, sp):
                    for k, (w, y) in enumerate(MMS_R):
                        nc.tensor.matmul(ps[0][ci][:], XT6[:, w, t], rsl(y, cs),
                                         start=st and k == 0, stop=sp and k == 5)
                    for k, (w, y) in enumerate(MMS_I):
                        nc.tensor.matmul(ps[1][ci][:], XT6[:, w, t], rsl(y, cs),
                                         start=st and k == 0, stop=sp and k == 5)

                if halves is not None:
                    for ci in range(NCH):
                        for t in range(KT):
                            rt = rpool.tile([P, 4, 512], bf16, tag="rhsh", name="rth", bufs=2)
                            nc.sync.dma_start(rt[:], halves[ci](t))
                            rsl = lambda y, cs, rt=rt: rt[:, y, :]
                            do_mms(ci, rsl, None, t == 0, t == KT - 1)
                else:
                    for t in range(KT):
                        if resident is None:
                            rt = rpool.tile([P, 4, N], bf16, tag="rhs", name="rt")
                            nc.sync.dma_start(rt[:], rhs_slice(t))
                            rsl = lambda y, cs: rt[:, y, cs]
                        else:
                            rsl = lambda y, cs, t=t: resident(y, t)[:, cs]
                        st = t == 0
                        sp = t == KT - 1
                        for ci in range(NCH):
                            cs = slice(512 * ci, 512 * ci + 512)
                            do_mms(ci, rsl, cs, st, sp)
                for j in range(2):
                    for ci in range(NCH):
                        out_cb(j, ci, ps[j][ci])

            def transpose_to_weights_sp(src, tag, pool=wrot, bufs=None):
                """fp32 [P, 2, N] shard tile -> [P,6,KT,P] split-bf16 weights."""
                wt = pool.tile([P, 6, KT, P], bf16, tag=tag, name="tsp_" + tag,
                               bufs=bufs)
                for j in range(2):
                    for t in range(KT):
                        tp = tppool.tile([P, P], f32, tag="tp", name="tp")
                        nc.tensor.transpose(tp[:], src[:, j, 128 * t:128 * t + 128], ident[:])
                        h = 2 * j
                        nc.vector.tensor_copy(wt[:, h, t], tp[:])
                        nc.vector.tensor_sub(wt[:, h + 1, t], tp[:], wt[:, h, t])
                        if j == 1:
                            nc.vector.tensor_scalar_mul(wt[:, 4, t], tp[:], -1.0)
                            nc.vector.tensor_sub(wt[:, 5, t], wt[:, 2, t], tp[:])
                return wt

            def transpose_to_weights(src, tag, dtype, pool=wrot, bufs=None):
                """[P, 2, N] shard tile -> [P,3,KT,P] transposed weights."""
                wt = pool.tile([P, 3, KT, P], dtype, tag=tag, name="tw_" + tag,
                               bufs=bufs)
                bf = src.dtype == bf16
                for j in range(2):
                    for t in range(KT):
                        tp = tppool.tile([P, P], bf16 if bf else f32, tag="tp", name="tp")
                        blk = src[:, j, 128 * t:128 * t + 128]
                        if bf:
                            nc.tensor.transpose(tp[:], blk, ident_bf[:])
                        else:
                            nc.tensor.transpose(tp[:], blk, ident[:])
                        nc.vector.tensor_copy(wt[:, j, t], tp[:])
                        if j == 1:
                            nc.vector.tensor_scalar_mul(wt[:, 2, t], tp[:], -1.0)
                return wt

            def cb_store(dst):
                def cb(j, ci, psum):
                    nc.vector.tensor_copy(dst[:, j, 512 * ci:512 * ci + 512], psum[:])
                return cb

            def allgather(ag_in, ag_out):
                nc.gpsimd.collective_compute(
                    "AllGather", mybir.AluOpType.bypass, replica_groups=RG,
                    ins=[ag_in.opt()], outs=[ag_out.opt()],
                )

            def src_of(dram_t):
                ap = dram_t.ap()
                return lambda j, t: ap[:, j, t]

            wsp_src = lambda y, t: Wsp[:, y, t]

            # ---------------- squares first: A1 = A^2, W1 = W^2 (bf16) ----
            # input-only deps, so their AllGather flies behind all of phase 1
            ATbf = load_weights(ATsh_bf, tag="T3", dtype=bf16, bufs=1)
            WTbf = load_weights(WTsh_bf, tag="T2", dtype=bf16, bufs=1)
            A1 = apool.tile([P, 2, N], bf16, tag="work", bufs=3, name="A1")
            cgemm(ATbf, src_of(Afull_bf), cb_store(A1), bf16)

            aga_in = dram.tile([2, P, N], bf16, name="aga_in")
            aga_out = dram.tile([NC, 2, P, N], bf16, addr_space="Shared", name="aga_out")
            W1 = apool.tile([P, 2, N], bf16, tag="work", bufs=3, name="W1")

            def cb_w1(j, ci, psum):
                cs = slice(512 * ci, 512 * ci + 512)
                nc.vector.tensor_copy(W1[:, j, cs], psum[:])
                nc.sync.dma_start(aga_in[j, :, cs], W1[:, j, cs])

            cgemm(WTbf, src_of(Wfull_bf), cb_w1, bf16)
            allgather(aga_in, aga_out)
            AT1 = transpose_to_weights(A1, tag="T3", dtype=bf16, bufs=1)

            # ---------------- phase 1 (no collective deps) ----------------
            CT32 = load_weights(CTsh32, tag="T2", dtype=f32, bufs=1)

            # V = A @ C  (split-bf16)
            V = apool.tile([P, 2, N], f32, tag="work", bufs=3, name="V")
            cspa = Csp.ap()
            cgemm_sp(ATspw, lambda t: cspa[:, :, t], cb_store(V))

            # R = C @ Y  (fp32, exact: R enters the answer directly)
            R32 = apool.tile([P, 2, N], f32, tag="R32", bufs=1)
            cgemm(CT32, src_of(Yfull32), cb_store(R32), f32)

            VT = transpose_to_weights_sp(V, tag="T1")

            # U0 = V @ Y  (split-bf16)
            U0 = apool.tile([P, 2, N], f32, tag="work", bufs=3, name="U0")
            yspa = Ysp.ap()
            cgemm_sp(VT, lambda t: yspa[:, :, t], cb_store(U0))
            U0T = transpose_to_weights_sp(U0, tag="T2", bufs=1)

            # S1 = R + U0 @ W  (W from SBUF; bf16 halves feed 2 AllGathers)
            S1 = apool.tile([P, 2, N], f32, tag="Sch", bufs=2, name="S1")
            agb_ins = [dram.tile([P, 2, 512], bf16, name="agb_in") for _ in range(NCH)]
            agb_outs = [dram.tile([NC, P, 2, 512], bf16, addr_space="Shared", name="agb_out")
                        for _ in range(NCH)]

            def cb_s1(j, ci, psum):
                cs = slice(512 * ci, 512 * ci + 512)
                nc.vector.tensor_add(S1[:, j, cs], psum[:], R32[:, j, cs])
                stg = apool.tile([P, 512], bf16, tag="stg", bufs=6, name="stg")
                nc.vector.tensor_add(stg[:], psum[:], R32[:, j, cs])
                nc.sync.dma_start(agb_ins[ci][:, j, :], stg[:])

            cgemm_sp(U0T, None, cb_s1, resident=wsp_src)
            for ci in range(NCH):
                allgather(agb_ins[ci], agb_outs[ci])

            # ---------------- step 2: S2 = S1 + A1 S1 W1 (bf16 GEMMs) -----
            U1 = apool.tile([P, 2, N], bf16, tag="work", bufs=3, name="U1")
            cgemm(AT1, None, cb_store(U1), bf16,
                  halves=[lambda t, ci=ci: agb_outs[ci][t] for ci in range(NCH)])
            U1T = transpose_to_weights(U1, tag="T2", dtype=bf16, bufs=1)

            S2 = apool.tile([P, 2, N], f32, tag="Sch", bufs=2, name="S2")
            agc_ins = [dram.tile([P, 4, 512], bf16, name="agc_in") for _ in range(NCH)]
            agc_outs = [dram.tile([NC, P, 4, 512], bf16, addr_space="Shared", name="agc_out")
                        for _ in range(NCH)]

            def mk_cb_split(Sdst, addend, ag_ins):
                def cb(j, ci, psum):
                    cs = slice(512 * ci, 512 * ci + 512)
                    nc.vector.tensor_add(Sdst[:, j, cs], psum[:], addend[:, j, cs])
                    h = apool.tile([P, 512], bf16, tag="stg", bufs=6, name="stgh")
                    l = apool.tile([P, 512], bf16, tag="stg", bufs=6, name="stgl")
                    nc.vector.tensor_copy(h[:], Sdst[:, j, cs])
                    nc.vector.tensor_sub(l[:], Sdst[:, j, cs], h[:])
                    nc.sync.dma_start(ag_ins[ci][:, 2 * j, :], h[:])
                    nc.sync.dma_start(ag_ins[ci][:, 2 * j + 1, :], l[:])
                return cb

            cgemm(U1T, lambda j, t: aga_out[t, j], mk_cb_split(S2, S1, agc_ins), bf16)
            for ci in range(NCH):
                allgather(agc_ins[ci], agc_outs[ci])

            # ---------------- polish x2 (split-bf16) ----------------------
            s_half = [lambda t, ci=ci: agc_outs[ci][t] for ci in range(NCH)]
            for it in range(2):
                last = it == 1
                Up = apool.tile([P, 2, N], f32, tag="work", bufs=3, name="Up")
                cgemm_sp(ATspw, None, cb_store(Up), halves=s_half)
                UpT = transpose_to_weights_sp(Up, tag="T1" if it == 0 else "T3",
                                              bufs=2 if it == 0 else 1)

                Sp = apool.tile([P, 2, N], f32, tag="Sch", bufs=2, name="Sp")
                if not last:
                    agd_ins = [dram.tile([P, 4, 512], bf16, name="agd_in") for _ in range(NCH)]
                    agd_outs = [dram.tile([NC, P, 4, 512], bf16, addr_space="Shared",
                                          name="agd_out") for _ in range(NCH)]
                    cgemm_sp(UpT, None, mk_cb_split(Sp, R32, agd_ins), resident=wsp_src)
                    for ci in range(NCH):
                        allgather(agd_ins[ci], agd_outs[ci])
                    s_half = [lambda t, ci=ci, agd_outs=agd_outs: agd_outs[ci][t]
                              for ci in range(NCH)]
                else:
                    def cb_fin(j, ci, psum):
                        cs = slice(512 * ci, 512 * ci + 512)
                        nc.vector.tensor_add(Sp[:, j, cs], psum[:], R32[:, j, cs])
                        nc.sync.dma_start(out.ap()[j, :, cs], Sp[:, j, cs])

                    cgemm_sp(UpT, None, cb_fin, resident=wsp_src)

    nc.compile()
    return nc


def _prep_inputs(A, W, C, Y):
    import ml_dtypes
    bf = ml_dtypes.bfloat16

    def full_layout(M, dt=np.float32):
        pl = np.stack([
            M.real.astype(np.float32).astype(dt),
            M.imag.astype(np.float32).astype(dt),
        ])  # [2, 1024, 1024]
        return np.ascontiguousarray(pl.reshape(2, KT, P, N).transpose(2, 0, 1, 3))

    def shard_weights(M, c, dt=np.float32):
        own = M[P * c:P * c + P, :]
        XT = own.T
        r = XT.real.astype(np.float32)
        i = XT.imag.astype(np.float32)
        tr = np.stack([r, i, -i]).astype(dt)  # [3, 1024, 128]
        return np.ascontiguousarray(tr.reshape(3, KT, P, P).transpose(2, 0, 1, 3))

    def split_layout(M):
        # [P, 4, KT, N] bf16: planes (re_h, re_l, im_h, im_l)
        planes = []
        for part in (M.real, M.imag):
            x = part.astype(np.float32)
            h = x.astype(bf)
            l = (x - h.astype(np.float32)).astype(bf)
            planes += [h, l]
        pl = np.stack(planes)  # [4, 1024, 1024]
        return np.ascontiguousarray(pl.reshape(4, KT, P, N).transpose(2, 0, 1, 3))

    def split_shard(M, c):
        # [P, 6, KT, P] bf16: (rh, rl, ih, il, -ih, -il) of own-shard transpose
        XT = M[P * c:P * c + P, :].T
        r = XT.real.astype(np.float32)
        i = XT.imag.astype(np.float32)
        rh = r.astype(bf); rl = (r - rh.astype(np.float32)).astype(bf)
        ih = i.astype(bf); il = (i - ih.astype(np.float32)).astype(bf)
        tr = np.stack([rh, rl, ih, il, -ih, -il])  # [6, 1024, 128]
        return np.ascontiguousarray(tr.reshape(6, KT, P, P).transpose(2, 0, 1, 3))

    Yf = full_layout(Y)
    Abf = full_layout(A, bf)
    Wbf = full_layout(W, bf)
    Cs = split_layout(C)
    Ys = split_layout(Y)
    Ws = split_layout(W)
    in_maps = []
    for c in range(NC):
        in_maps.append({
            "Csp": Cs, "Ysp": Ys, "Wsp_d": Ws, "Yfull32": Yf,
            "Afull_bf": Abf, "Wfull_bf": Wbf,
            "ATsp": split_shard(A, c),
            "ATsh_bf": shard_weights(A, c, bf),
            "WTsh_bf": shard_weights(W, c, bf),
            "CTsh32": shard_weights(C, c),
        })
    return in_maps


def kernel(A, W, C, Y, _trace=False):
    from concourse import bass_utils

    if "nc" not in _compiled:
        _compiled["nc"] = _build()
    nc = _compiled["nc"]

    in_maps = _prep_inputs(A, W, C, Y)
    res = bass_utils.run_bass_kernel_spmd(
        nc, in_maps, core_ids=list(range(NC)), trace=_trace
    )
    _compiled["last_result"] = res

    full = np.empty((N, N), dtype=np.complex128)
    for c in range(NC):
        o = res.results[c]["out"]
        full[P * c:P * c + P, :] = o[0].astype(np.float64) + 1j * o[1].astype(np.float64)
    return full


# revision 21
# speedup vs baseline: 1.0769x; 1.0769x over previous
"""Stein solver  Lambda - A @ Lambda @ W = C @ Y  on 8 trn2 NeuronCores.

Math: Lambda = sum_k A^k R W^k with R = C@Y; contraction ||A||2*||W||2 ~ 0.32.
Smith doubling truncated at 4 terms, then 2 exact fixed-point iterations:
    S1 = R + (A C) Y W           (2 terms; U0 = (A C) Y avoids gathering R)
    S2 = S1 + A^2 S1 W^2         (4 terms; ~1%-magnitude term, bf16 GEMMs)
    Sp = R + A S W   (x2, fp32)  (polish: each contracts error ~12x -> ~5e-7)

Distribution: row-sharded over 8 cores, core c owns rows [128c, 128c+128).
Stationary operand = transposed own-shard (8 k-tiles of [128,128]); moving
operand = the full matrix, streamed from DRAM -- except W, which stays
resident in SBUF (64KB/partition) and feeds three cgemms for free.
AllGathers carry only what later phases read as full matrices: S1 (bf16,
hidden behind the A^2/W^2 cgemms), W^2 (bf16), S2 (fp32), Sp (fp32).

Precision: fp32 GEMMs (4 cyc/row) everywhere on the main chain -- the fp32
phases are DMA-bound anyway, so the extra PE cycles are free and keep the
PE warm; bf16 GEMMs (1 cyc/row, half DMA) for the 1%-scale S2 update, whose
error is wiped by the polish.  Complex GEMM = 4 real GEMMs with the real-part
subtraction folded into PSUM accumulation via pre-negated imag weights.
"""

import numpy as np

P = 128
N = 1024
KT = N // P          # 8 k-tiles
NC = 8               # cores
NCH = 2              # 512-wide n-chunks per 1024-col output row block

_compiled = {}


def _build():
    import concourse.mybir as mybir
    import concourse.tile as tile
    from concourse import bacc
    from concourse.masks import make_identity

    f32 = mybir.dt.float32
    bf16 = mybir.dt.bfloat16

    nc = bacc.Bacc("TRN2", target_bir_lowering=False, debug=False, num_devices=NC)

    # ---- I/O ----  full matrices laid out [partition, plane, ktile, col]:
    # X[kt*128+p, c] at [p, j, kt, c]; shards [partition, (re,im,-im), ktile, m]
    f32r = mybir.dt.float32r
    Cfull = nc.dram_tensor("Cfull", [P, 2, KT, N], f32r, kind="ExternalInput")
    Wfull = nc.dram_tensor("Wfull", [P, 2, KT, N], f32r, kind="ExternalInput")
    ATshr = nc.dram_tensor("ATshr", [P, 3, KT, P], f32r, kind="ExternalInput")
    Wsp_d = nc.dram_tensor("Wsp_d", [P, 4, KT, N], bf16, kind="ExternalInput")
    Yfull32 = nc.dram_tensor("Yfull32", [P, 2, KT, N], f32, kind="ExternalInput")
    Afull_bf = nc.dram_tensor("Afull_bf", [P, 2, KT, N], bf16, kind="ExternalInput")
    Wfull_bf = nc.dram_tensor("Wfull_bf", [P, 2, KT, N], bf16, kind="ExternalInput")
    ATsp = nc.dram_tensor("ATsp", [P, 6, KT, P], bf16, kind="ExternalInput")
    ATsh_bf = nc.dram_tensor("ATsh_bf", [P, 3, KT, P], bf16, kind="ExternalInput")
    WTsh_bf = nc.dram_tensor("WTsh_bf", [P, 3, KT, P], bf16, kind="ExternalInput")
    CTsh32 = nc.dram_tensor("CTsh32", [P, 3, KT, P], f32, kind="ExternalInput")
    out = nc.dram_tensor("out", [2, P, N], f32, kind="ExternalOutput")

    RG = [list(range(NC))]

    with tile.TileContext(nc) as tc:
        with (
            tc.tile_pool(name="wpin", bufs=1) as wpin,        # pinned
            tc.tile_pool(name="wrot", bufs=2) as wrot,        # rotating weights
            tc.tile_pool(name="rhs", bufs=2) as rpool,        # rhs stream tiles
            tc.tile_pool(name="acc", bufs=2) as apool,        # shard accumulators
            tc.tile_pool(name="psum", bufs=6, space="PSUM") as ppool,
            tc.tile_pool(name="tpsum", bufs=2, space="PSUM") as tppool,
            tc.tile_pool(name="dram", bufs=1, space="DRAM") as dram,
        ):
            ident = wpin.tile([P, P], f32, tag="ident")
            make_identity(nc, ident)
            ident_bf = wpin.tile([P, P], bf16, tag="identbf")
            nc.vector.tensor_copy(ident_bf[:], ident[:])

            # W (split hi/lo bf16) stays SBUF-resident: feeds S1/Sp cgemms.
            Wsp = wpin.tile([P, 4, KT, N], bf16, tag="Wsp")
            nc.sync.dma_start(Wsp[:], Wsp_d.ap())
            ATspw = wpin.tile([P, 6, KT, P], bf16, tag="ATspw")
            nc.sync.dma_start(ATspw[:], ATsp.ap())

            def load_weights(dram_t, tag, dtype, pool=wrot, bufs=None):
                wt = pool.tile([P, 3, KT, P], dtype, tag=tag, name="wt_" + tag,
                               bufs=bufs)
                nc.sync.dma_start(wt[:], dram_t.ap())
                return wt

            def cgemm(XT, rhs_slice, out_cb, dtype, resident=None, halves=None):
                """out(own 128 rows x 1024, complex) = own_rows(X) @ M.

                XT: [P,3,KT,P] weights (re, im, -im).  Moving operand: either
                rhs_slice(j,t) -> DRAM [P,N] AP (streamed via SBUF tiles) or
                resident(j,t) -> SBUF [P,N] AP.  out_cb(j, ci, psum).
                """
                ps = [[ppool.tile([P, 512], f32, tag="ps", name="ps")
                       for _ in range(NCH)] for _ in range(2)]

                def do4(ci, rsl, cs, st, sp):
                    nc.tensor.matmul(ps[0][ci][:], XT[:, 0, t], rsl(0, cs), start=st, stop=False)
                    nc.tensor.matmul(ps[0][ci][:], XT[:, 2, t], rsl(1, cs), start=False, stop=sp)
                    nc.tensor.matmul(ps[1][ci][:], XT[:, 0, t], rsl(1, cs), start=st, stop=False)
                    nc.tensor.matmul(ps[1][ci][:], XT[:, 1, t], rsl(0, cs), start=False, stop=sp)

                if halves is not None:
                    for ci in range(NCH):
                        for t in range(KT):
                            rt = rpool.tile([P, 2, 512], dtype, tag="rhsh", name="rth", bufs=2)
                            nc.sync.dma_start(rt[:], halves[ci](t))
                            do4(ci, (lambda j, cs, rt=rt: rt[:, j, :]), None, t == 0, t == KT - 1)
                else:
                    for t in range(KT):
                        if resident is None:
                            rt = rpool.tile([P, 2, N], dtype, tag="rhs", name="rt")
                            nc.sync.dma_start(rt[:, 0], rhs_slice(0, t))
                            nc.sync.dma_start(rt[:, 1], rhs_slice(1, t))
                            rsl = lambda j, cs: rt[:, j, cs]
                        else:
                            rsl = lambda j, cs, t=t: resident(j, t)[:, cs]
                        st = t == 0
                        sp = t == KT - 1
                        for ci in range(NCH):
                            cs = slice(512 * ci, 512 * ci + 512)
                            do4(ci, rsl, cs, st, sp)
                for j in range(2):
                    for ci in range(NCH):
                        out_cb(j, ci, ps[j][ci])

            def cgemm_sp(XT6, rhs_slice, out_cb, resident=None, halves=None):
                """Split-bf16 complex GEMM: X, M given as hi/lo bf16 pairs.

                XT6: [P,6,KT,P] weights (rh, rl, ih, il, -ih, -il).
                rhs_slice(t) -> DRAM [P,4,N] AP (planes rh, rl, ih, il), or
                resident(y, t) -> SBUF [P,N] AP, or halves[ci](t) -> [P,4,512]
                column-half APs (chunk-outer order, for pipelined AllGathers).
                """
                MMS_R = ((0, 0), (0, 1), (1, 0), (4, 2), (4, 3), (5, 2))
                MMS_I = ((0, 2), (0, 3), (1, 2), (2, 0), (2, 1), (3, 0))
                ps = [[ppool.tile([P, 512], f32, tag="ps", name="ps")
                       for _ in range(NCH)] for _ in range(2)]

                def do_mms(ci, rsl, cs, st, sp):
                    for k, (w, y) in enumerate(MMS_R):
                        nc.tensor.matmul(ps[0][ci][:], XT6[:, w, t], rsl(y, cs),
                                         start=st and k == 0, stop=sp and k == 5)
                    for k, (w, y) in enumerate(MMS_I):
                        nc.tensor.matmul(ps[1][ci][:], XT6[:, w, t], rsl(y, cs),
                                         start=st and k == 0, stop=sp and k == 5)

                if halves is not None:
                    for ci in range(NCH):
                        for t in range(KT):
                            rt = rpool.tile([P, 4, 512], bf16, tag="rhsh", name="rth", bufs=2)
                            nc.sync.dma_start(rt[:], halves[ci](t))
                            rsl = lambda y, cs, rt=rt: rt[:, y, :]
                            do_mms(ci, rsl, None, t == 0, t == KT - 1)
                else:
                    for t in range(KT):
                        if resident is None:
                            rt = rpool.tile([P, 4, N], bf16, tag="rhs", name="rt")
                            nc.sync.dma_start(rt[:], rhs_slice(t))
                            rsl = lambda y, cs: rt[:, y, cs]
                        else:
                            rsl = lambda y, cs, t=t: resident(y, t)[:, cs]
                        st = t == 0
                        sp = t == KT - 1
                        for ci in range(NCH):
                            cs = slice(512 * ci, 512 * ci + 512)
                            do_mms(ci, rsl, cs, st, sp)
                for j in range(2):
                    for ci in range(NCH):
                        out_cb(j, ci, ps[j][ci])

            def transpose_to_weights_sp(src, tag, pool=wrot, bufs=None):
                """fp32 [P, 2, N] shard tile -> [P,6,KT,P] split-bf16 weights."""
                wt = pool.tile([P, 6, KT, P], bf16, tag=tag, name="tsp_" + tag,
                               bufs=bufs)
                for j in range(2):
                    for t in range(KT):
                        tp = tppool.tile([P, P], f32, tag="tp", name="tp")
                        nc.tensor.transpose(tp[:], src[:, j, 128 * t:128 * t + 128], ident[:])
                        h = 2 * j
                        nc.vector.tensor_copy(wt[:, h, t], tp[:])
                        nc.vector.tensor_sub(wt[:, h + 1, t], tp[:], wt[:, h, t])
                        if j == 1:
                            nc.vector.tensor_scalar_mul(wt[:, 4, t], tp[:], -1.0)
                            nc.vector.tensor_sub(wt[:, 5, t], wt[:, 2, t], tp[:])
                return wt

            def transpose_to_weights(src, tag, dtype, pool=wrot, bufs=None):
                """[P, 2, N] shard tile -> [P,3,KT,P] transposed weights."""
                wt = pool.tile([P, 3, KT, P], dtype, tag=tag, name="tw_" + tag,
                               bufs=bufs)
                bf = src.dtype == bf16
                for j in range(2):
                    for t in range(KT):
                        tp = tppool.tile([P, P], bf16 if bf else f32, tag="tp", name="tp")
                        blk = src[:, j, 128 * t:128 * t + 128]
                        if bf:
                            nc.tensor.transpose(tp[:], blk, ident_bf[:])
                        else:
                            nc.tensor.transpose(tp[:], blk, ident[:])
                        nc.vector.tensor_copy(wt[:, j, t], tp[:])
                        if j == 1:
                            nc.vector.tensor_scalar_mul(wt[:, 2, t], tp[:], -1.0)
                return wt

            def cb_store(dst):
                def cb(j, ci, psum):
                    nc.vector.tensor_copy(dst[:, j, 512 * ci:512 * ci + 512], psum[:])
                return cb

            def allgather(ag_in, ag_out):
                nc.gpsimd.collective_compute(
                    "AllGather", mybir.AluOpType.bypass, replica_groups=RG,
                    ins=[ag_in.opt()], outs=[ag_out.opt()],
                )

            def src_of(dram_t):
                ap = dram_t.ap()
                return lambda j, t: ap[:, j, t]

            wsp_src = lambda y, t: Wsp[:, y, t]

            # ---------------- squares first: A1 = A^2, W1 = W^2 (bf16) ----
            # input-only deps, so their AllGather flies behind all of phase 1
            ATbf = load_weights(ATsh_bf, tag="T3", dtype=bf16, bufs=1)
            WTbf = load_weights(WTsh_bf, tag="T2", dtype=bf16, bufs=1)
            A1 = apool.tile([P, 2, N], bf16, tag="work", bufs=3, name="A1")
            cgemm(ATbf, src_of(Afull_bf), cb_store(A1), bf16)

            aga_in = dram.tile([2, P, N], bf16, name="aga_in")
            aga_out = dram.tile([NC, 2, P, N], bf16, addr_space="Shared", name="aga_out")
            W1 = apool.tile([P, 2, N], bf16, tag="work", bufs=3, name="W1")

            def cb_w1(j, ci, psum):
                cs = slice(512 * ci, 512 * ci + 512)
                nc.vector.tensor_copy(W1[:, j, cs], psum[:])
                nc.sync.dma_start(aga_in[j, :, cs], W1[:, j, cs])

            cgemm(WTbf, src_of(Wfull_bf), cb_w1, bf16)
            allgather(aga_in, aga_out)
            AT1 = transpose_to_weights(A1, tag="T3", dtype=bf16, bufs=1)

            # ---------------- phase 1 (no collective deps) ----------------
            CT32 = load_weights(CTsh32, tag="T2", dtype=f32, bufs=1)

            # V = A @ C  (f32r)
            ATw = load_weights(ATshr, tag="T1", dtype=f32r)
            V = apool.tile([P, 2, N], f32, tag="work", bufs=3, name="V")
            cgemm(ATw, src_of(Cfull), cb_store(V), f32r)

            # R = C @ Y  (fp32, exact: R enters the answer directly)
            R32 = apool.tile([P, 2, N], f32, tag="R32", bufs=1)
            cgemm(CT32, src_of(Yfull32), cb_store(R32), f32)

            VT = transpose_to_weights(V, tag="T1", dtype=f32r)

            # U0 = V @ Y  (f32r; Y bytes are fp32, the PE rounds on read)
            U0 = apool.tile([P, 2, N], f32, tag="work", bufs=3, name="U0")
            ysrc = src_of(Yfull32)
            cgemm(VT, lambda j, t: ysrc(j, t).bitcast(f32r), cb_store(U0), f32r)
            U0T = transpose_to_weights(U0, tag="T2", dtype=f32r, bufs=1)

            # S1 = R + U0 @ W  (W from SBUF; bf16 halves feed 2 AllGathers)
            S1 = apool.tile([P, 2, N], f32, tag="Sch", bufs=2, name="S1")
            agb_ins = [dram.tile([P, 2, 512], bf16, name="agb_in") for _ in range(NCH)]
            agb_outs = [dram.tile([NC, P, 2, 512], bf16, addr_space="Shared", name="agb_out")
                        for _ in range(NCH)]

            def cb_s1(j, ci, psum):
                cs = slice(512 * ci, 512 * ci + 512)
                nc.vector.tensor_add(S1[:, j, cs], psum[:], R32[:, j, cs])
                stg = apool.tile([P, 512], bf16, tag="stg", bufs=6, name="stg")
                nc.vector.tensor_add(stg[:], psum[:], R32[:, j, cs])
                nc.sync.dma_start(agb_ins[ci][:, j, :], stg[:])

            cgemm(U0T, src_of(Wfull), cb_s1, f32r)
            for ci in range(NCH):
                allgather(agb_ins[ci], agb_outs[ci])

            # ---------------- step 2: S2 = S1 + A1 S1 W1 (bf16 GEMMs) -----
            U1 = apool.tile([P, 2, N], bf16, tag="work", bufs=3, name="U1")
            cgemm(AT1, None, cb_store(U1), bf16,
                  halves=[lambda t, ci=ci: agb_outs[ci][t] for ci in range(NCH)])
            U1T = transpose_to_weights(U1, tag="T2", dtype=bf16, bufs=1)

            S2 = apool.tile([P, 2, N], f32, tag="Sch", bufs=2, name="S2")
            agc_ins = [dram.tile([P, 4, 512], bf16, name="agc_in") for _ in range(NCH)]
            agc_outs = [dram.tile([NC, P, 4, 512], bf16, addr_space="Shared", name="agc_out")
                        for _ in range(NCH)]

            def mk_cb_split(Sdst, addend, ag_ins):
                def cb(j, ci, psum):
                    cs = slice(512 * ci, 512 * ci + 512)
                    nc.vector.tensor_add(Sdst[:, j, cs], psum[:], addend[:, j, cs])
                    h = apool.tile([P, 512], bf16, tag="stg", bufs=6, name="stgh")
                    l = apool.tile([P, 512], bf16, tag="stg", bufs=6, name="stgl")
                    nc.vector.tensor_copy(h[:], Sdst[:, j, cs])
                    nc.vector.tensor_sub(l[:], Sdst[:, j, cs], h[:])
                    nc.sync.dma_start(ag_ins[ci][:, 2 * j, :], h[:])
                    nc.sync.dma_start(ag_ins[ci][:, 2 * j + 1, :], l[:])
                return cb

            cgemm(U1T, lambda j, t: aga_out[t, j], mk_cb_split(S2, S1, agc_ins), bf16)
            for ci in range(NCH):
                allgather(agc_ins[ci], agc_outs[ci])

            # ---------------- polish x2 (split-bf16) ----------------------
            s_half = [lambda t, ci=ci: agc_outs[ci][t] for ci in range(NCH)]
            for it in range(2):
                last = it == 1
                Up = apool.tile([P, 2, N], f32, tag="work", bufs=3, name="Up")
                cgemm_sp(ATspw, None, cb_store(Up), halves=s_half)
                UpT = transpose_to_weights_sp(Up, tag="T1" if it == 0 else "T3",
                                              bufs=2 if it == 0 else 1)

                Sp = apool.tile([P, 2, N], f32, tag="Sch", bufs=2, name="Sp")
                if not last:
                    agd_ins = [dram.tile([P, 4, 512], bf16, name="agd_in") for _ in range(NCH)]
                    agd_outs = [dram.tile([NC, P, 4, 512], bf16, addr_space="Shared",
                                          name="agd_out") for _ in range(NCH)]
                    cgemm_sp(UpT, None, mk_cb_split(Sp, R32, agd_ins), resident=wsp_src)
                    for ci in range(NCH):
                        allgather(agd_ins[ci], agd_outs[ci])
                    s_half = [lambda t, ci=ci, agd_outs=agd_outs: agd_outs[ci][t]
                              for ci in range(NCH)]
                else:
                    def cb_fin(j, ci, psum):
                        cs = slice(512 * ci, 512 * ci + 512)
                        nc.vector.tensor_add(Sp[:, j, cs], psum[:], R32[:, j, cs])
                        nc.sync.dma_start(out.ap()[j, :, cs], Sp[:, j, cs])

                    cgemm_sp(UpT, None, cb_fin, resident=wsp_src)

    nc.compile()
    return nc


def _prep_inputs(A, W, C, Y):
    import ml_dtypes
    bf = ml_dtypes.bfloat16

    def full_layout(M, dt=np.float32):
        pl = np.stack([
            M.real.astype(np.float32).astype(dt),
            M.imag.astype(np.float32).astype(dt),
        ])  # [2, 1024, 1024]
        return np.ascontiguousarray(pl.reshape(2, KT, P, N).transpose(2, 0, 1, 3))

    def shard_weights(M, c, dt=np.float32):
        own = M[P * c:P * c + P, :]
        XT = own.T
        r = XT.real.astype(np.float32)
        i = XT.imag.astype(np.float32)
        tr = np.stack([r, i, -i]).astype(dt)  # [3, 1024, 128]
        return np.ascontiguousarray(tr.reshape(3, KT, P, P).transpose(2, 0, 1, 3))

    def split_layout(M):
        # [P, 4, KT, N] bf16: planes (re_h, re_l, im_h, im_l)
        planes = []
        for part in (M.real, M.imag):
            x = part.astype(np.float32)
            h = x.astype(bf)
            l = (x - h.astype(np.float32)).astype(bf)
            planes += [h, l]
        pl = np.stack(planes)  # [4, 1024, 1024]
        return np.ascontiguousarray(pl.reshape(4, KT, P, N).transpose(2, 0, 1, 3))

    def split_shard(M, c):
        # [P, 6, KT, P] bf16: (rh, rl, ih, il, -ih, -il) of own-shard transpose
        XT = M[P * c:P * c + P, :].T
        r = XT.real.astype(np.float32)
        i = XT.imag.astype(np.float32)
        rh = r.astype(bf); rl = (r - rh.astype(np.float32)).astype(bf)
        ih = i.astype(bf); il = (i - ih.astype(np.float32)).astype(bf)
        tr = np.stack([rh, rl, ih, il, -ih, -il])  # [6, 1024, 128]
        return np.ascontiguousarray(tr.reshape(6, KT, P, P).transpose(2, 0, 1, 3))

    Yf = full_layout(Y)
    Cf = full_layout(C)
    Wf = full_layout(W)
    Abf = full_layout(A, bf)
    Wbf = full_layout(W, bf)
    Ws = split_layout(W)
    in_maps = []
    for c in range(NC):
        ATs = shard_weights(A, c)
        in_maps.append({
            "Cfull": Cf, "Wfull": Wf, "Wsp_d": Ws, "Yfull32": Yf,
            "Afull_bf": Abf, "Wfull_bf": Wbf,
            "ATshr": ATs,
            "ATsp": split_shard(A, c),
            "ATsh_bf": shard_weights(A, c, bf),
            "WTsh_bf": shard_weights(W, c, bf),
            "CTsh32": shard_weights(C, c),
        })
    return in_maps


def kernel(A, W, C, Y, _trace=False):
    from concourse import bass_utils

    if "nc" not in _compiled:
        _compiled["nc"] = _build()
    nc = _compiled["nc"]

    in_maps = _prep_inputs(A, W, C, Y)
    res = bass_utils.run_bass_kernel_spmd(
        nc, in_maps, core_ids=list(range(NC)), trace=_trace
    )
    _compiled["last_result"] = res

    full = np.empty((N, N), dtype=np.complex128)
    for c in range(NC):
        o = res.results[c]["out"]
        full[P * c:P * c + P, :] = o[0].astype(np.float64) + 1j * o[1].astype(np.float64)
    return full
